# revision 53
# baseline (speedup 1.0000x reference)
"""EdgeUpdate (gnn_message_passing) Trainium2 Bass kernel — 8 NeuronCores.

Contract: kernel(**inputs) takes the FULL inputs of reference.setup_inputs()
and returns the FULL [32768, 64] float32 output.

Strategy (edge-parallel SPMD):
- Edges are sharded 8 ways (4096 edges/core); node features and all
  FFN/LN parameters are replicated. One NEFF runs on cores 0-7 with
  per-core input bindings. No cross-core communication.

Per-core kernel (tiles of 128 edges, edges on partitions). DVE is the
bottleneck engine (~89% busy: the per-edge tensor-product contraction
runs as fused multiply-scan passes over the FFN2 output), so everything
else is kept off DVE and off its critical path:
- All inputs are SBUF-resident, loaded up front: tile-0-critical slices
  first on the sync queue (efsi tile 0, W1, eft cols 0:128, then b1),
  W2 block 0 concurrently on the Act HWDGE queue, bulk after. Residents
  are split into per-load-group tiles because dependency tracking is
  per-tile — one big tile would stall tile 0 on the bulk DMA. Outputs
  accumulate in a resident buffer and ship in 2-tile stores (small
  groups keep the final store off the drain path).
- The Act function table is warmed by a dummy Sqrt at t=0 so tile 0's
  relu doesn't wait ~2.6us for the table load.
- FFN1 computed transposed (rhT = relu(W1.T @ efT + b1)), bias per
  partition; eft is host-pretransposed.
- FFN2 (per-edge weight generation) in float32r into 16-h-page PSUM
  chunks, W2 host-permuted h-major.
- Chunks 0-2 are evacuated PSUM->SBUF bf16 by the Act engine; the DVE
  MUL_SCAN (running prefix of Src0*Src1, page-end sampled via a stride-0
  output dim) then reads SBUF (58-cycle access vs 120 for PSUM). Chunk 3
  scans straight from PSUM (Act can't absorb a 4th evacuation), as does
  all of tile 0 (no evac dependency -> DVE starts ~4us earlier).
- feats build on Pool (tensor_scalar with per-partition sh scalars).
- Tails are software-pipelined: the Pool x-assembly of tile t-1 is
  emitted before tile t's feats; bn_stats/bn_aggr (DVE) after tile t's
  scans; sqrt (Act) + reciprocal + LN affine (Pool) one tile later
  still. GPSIMD/Pool cannot touch PSUM, so the featsT/seed evacuations
  stay on Act.
"""

from contextlib import ExitStack

import numpy as np

import concourse.bass as bass
import concourse.dve_ops as dve_ops
import concourse.tile as tile
from concourse import bacc, mybir
from concourse.bass_utils import run_bass_kernel_spmd
from concourse.dve_spec import AluOp, C0, Spec, Src0, Src1, lower, scan
from concourse.dve_uop import DveOpSpec
from concourse.masks import make_identity

F32 = mybir.dt.float32
F32R = mybir.dt.float32r
BF16 = mybir.dt.bfloat16
N_CORES = 8
E_TOTAL = 32768
E_CORE = E_TOTAL // N_CORES
N_NODES = 16384
RES_DIM = 56
H = 64
W_IN = 80
W_NUMEL = 5120
EPS = 1e-5
CHUNK_PAGES = 16  # h-pages (of 80 values) per PSUM chunk
EFSI_W = H + 4 + 2 * RES_DIM  # 180


def _register_mul_scan():
    name = "MUL_SCAN_ANT"
    for op in dve_ops.OPS:
        if op.name == name:
            return op
    spec = Spec(
        body=scan(AluOp.ADD, Src0 * Src1, init=C0),
        reference=lambda in0, in1, s0, *a: (
            np.asarray(s0, np.float32)
            + np.cumsum(
                (in0.reshape(in0.shape[0], -1).astype(np.float32)
                 * in1.reshape(in1.shape[0], -1).astype(np.float32)),
                axis=-1)).astype(np.float32),
    )
    opcode = dve_ops._CUSTOM_DVE_ROW_BASE + len(dve_ops.OPS)
    shas = {}
    for ver in ("v3", "v4"):
        shas[ver] = DveOpSpec(name=name, opcode=opcode,
                              uops=lower(spec, ver=ver), rd1_en=True).sha(ver)
    op = dve_ops.DveOp(name, spec, subdim=False, uops_sha=shas)
    dve_ops.OPS.append(op)
    dve_ops.CUSTOM_DVE_SPECS[name] = spec
    dve_ops._SUB_OPCODE_FOR_NAME[name] = opcode
    return op


MUL_SCAN = _register_mul_scan()


PSUM_SCAN_CI = 3   # which chunk scans straight from PSUM (no Act evac)
SEED_AFTER = True  # emit the b2-seed path after the chunk loop
TILE0_PSUM = True  # tile 0 scans all chunks from PSUM (fast start)


def _build_kernel():
    n_tiles = E_CORE // 128
    chunks = [(0, 16), (16, 16), (32, 16), (48, 16)]

    nc = bacc.Bacc("TRN2", target_bir_lowering=False, debug=False,
                   enable_asserts=False, num_devices=N_CORES)

    # efsi row: [ef 64 | sh 4 | src 56 | dst 56] (src/dst host-gathered)
    efsi_ap = nc.dram_tensor("efsi", [E_CORE, EFSI_W], F32,
                             kind="ExternalInput").ap()
    eft_ap = nc.dram_tensor("eft", [H, E_CORE], BF16, kind="ExternalInput").ap()
    w1_ap = nc.dram_tensor("w1", [H, 128], BF16, kind="ExternalInput").ap()
    b1_ap = nc.dram_tensor("b1", [128, 1], F32, kind="ExternalInput").ap()
    w2_ap = nc.dram_tensor("w2", [128, W_NUMEL], BF16, kind="ExternalInput").ap()
    b2r_ap = nc.dram_tensor("b2r", [W_IN, H], F32, kind="ExternalInput").ap()
    gb_ap = nc.dram_tensor("gb", [2, H], F32, kind="ExternalInput").ap()
    out_ap = nc.dram_tensor("out", [E_CORE, H], F32, kind="ExternalOutput").ap()

    with tile.TileContext(nc) as tc, ExitStack() as ctx:
        singles = ctx.enter_context(tc.tile_pool(name="singles", bufs=1))
        mids = ctx.enter_context(tc.tile_pool(name="mids", bufs=3))
        wsb = ctx.enter_context(tc.tile_pool(name="wsb", bufs=2))
        outs = ctx.enter_context(tc.tile_pool(name="outs", bufs=3))
        ps_small = ctx.enter_context(
            tc.tile_pool(name="ps_small", bufs=1, space="PSUM"))
        ps_rh = ctx.enter_context(
            tc.tile_pool(name="ps_rh", bufs=1, space="PSUM"))
        ps_w = ctx.enter_context(tc.tile_pool(name="ps_w", bufs=2, space="PSUM"))

        # --- resident constants + inputs (all DMAs on the sync queue) ---
        # Warm the activation function table immediately (Sqrt forces the
        # sqrt_and_others set, which also holds Relu/Copy) so tile 0's relu
        # doesn't wait ~2.6us for the table load.
        epsb = singles.tile([128, 1], F32)
        nc.vector.memset(epsb[:], EPS)
        actwarm = singles.tile([128, 1], F32)
        nc.scalar.activation(actwarm[:], epsb[:],
                             mybir.ActivationFunctionType.Sqrt)

        w1sb = singles.tile([H, 128], BF16)
        b1sb = singles.tile([128, 1], F32)

        # Residents are split per load-group: the tile framework tracks
        # dependencies per-tile, so a single big tile would stall tile 0 on
        # the bulk DMA.
        EFSI_GROUPS = [(0, 1), (1, 4), (4, n_tiles)]
        EFT_GROUPS = [(0, 128), (128, 1024), (1024, E_CORE)]
        efsi_tiles = [singles.tile([128, (t1 - t0) * EFSI_W], F32,
                                   name=f"efsi{t0}", tag=f"efsi{t0}")
                      for t0, t1 in EFSI_GROUPS]
        eft_tiles = [singles.tile([H, c1 - c0], BF16,
                                  name=f"eft{c0}", tag=f"eft{c0}")
                     for c0, c1 in EFT_GROUPS]

        def efsi_slice(t):
            for (t0, t1), tl in zip(EFSI_GROUPS, efsi_tiles):
                if t0 <= t < t1:
                    return tl[:, (t - t0) * EFSI_W:(t - t0 + 1) * EFSI_W]
            raise AssertionError

        def eft_slice(t):
            for (c0, c1), tl in zip(EFT_GROUPS, eft_tiles):
                if c0 <= t * 128 < c1:
                    return tl[:, t * 128 - c0:(t + 1) * 128 - c0]
            raise AssertionError

        def efsi_load(gi):
            (t0, t1), tl = EFSI_GROUPS[gi], efsi_tiles[gi]
            dst = bass.AP(
                tensor=tl[:].tensor, offset=tl[:].offset,
                ap=[tl[:].ap[0], [EFSI_W, t1 - t0], [1, EFSI_W]])
            src = bass.AP(
                tensor=efsi_ap.tensor,
                offset=efsi_ap.offset + t0 * 128 * EFSI_W,
                ap=[[EFSI_W, 128], [128 * EFSI_W, t1 - t0], [1, EFSI_W]])
            nc.sync.dma_start(dst, src)

        w2blocks = []
        nblk = 4
        blkw = W_NUMEL // nblk

        def w2_load(bi, eng):
            w2b = singles.tile([128, blkw], BF16, tag=f"w2b{bi}")
            eng.dma_start(w2b[:], w2_ap[:, bi * blkw:(bi + 1) * blkw])
            w2blocks.append(w2b)

        # tile-0-critical loads first on the sync queue; w2b0 on the Act
        # queue — the two HWDGE queues run concurrently.
        efsi_load(0)
        nc.sync.dma_start(w1sb[:], w1_ap[:])
        nc.sync.dma_start(eft_tiles[0][:], eft_ap[:, 0:128])
        w2_load(0, nc.scalar)
        nc.sync.dma_start(b1sb[:], b1_ap[:])
        for bi in range(1, nblk):
            w2_load(bi, nc.sync)
        efsi_load(1)
        nc.sync.dma_start(eft_tiles[1][:], eft_ap[:, 128:1024])
        nc.sync.dma_start(eft_tiles[2][:], eft_ap[:, 1024:])
        efsi_load(2)

        b2rsb = singles.tile([W_IN, H], F32)
        nc.sync.dma_start(b2rsb[:], b2r_ap[:])
        gammab = singles.tile([128, H], F32)
        nc.sync.dma_start(gammab[:], bass.AP(
            tensor=gb_ap.tensor, offset=gb_ap.offset,
            ap=[[0, 128]] + gb_ap[0:1, :].ap[1:]))
        betab = singles.tile([128, H], F32)
        nc.sync.dma_start(betab[:], bass.AP(
            tensor=gb_ap.tensor, offset=gb_ap.offset + H,
            ap=[[0, 128]] + gb_ap[1:2, :].ap[1:]))
        ident = singles.tile([128, 128], F32)
        make_identity(nc, ident[:])
        y_r = singles.tile([128, n_tiles * H], F32)

        def w2_slice(c0, c1):
            bi = c0 // blkw
            assert c1 <= (bi + 1) * blkw
            return w2blocks[bi][:, c0 - bi * blkw:c1 - bi * blkw]

        def emit_tail1_pool(it, S, x_pre):
            # T = per-page sums from the unchained prefix samples: diff along
            # h, then add back the previous chunk's final prefix at the 3
            # chunk-boundary columns.
            T = outs.tile([128, H], F32, tag="T")
            nc.gpsimd.tensor_copy(T[:, 0:1], S[:, 0:1])
            nc.gpsimd.tensor_tensor(out=T[:, 1:], in0=S[:, 1:],
                                    in1=S[:, :H - 1],
                                    op=mybir.AluOpType.subtract)
            tb = bass.AP(tensor=T[:].tensor, offset=T[:].offset + CHUNK_PAGES,
                         ap=[T[:].ap[0], [CHUNK_PAGES, 3]])
            sb = bass.AP(tensor=S[:].tensor,
                         offset=S[:].offset + CHUNK_PAGES - 1,
                         ap=[S[:].ap[0], [CHUNK_PAGES, 3]])
            nc.gpsimd.tensor_tensor(out=tb, in0=tb, in1=sb,
                                    op=mybir.AluOpType.add)
            x = outs.tile([128, H], F32, tag="x")
            nc.gpsimd.tensor_tensor(out=x[:], in0=x_pre[:], in1=T[:],
                                    op=mybir.AluOpType.add)
            return it, x

        def emit_tail1_dve(it, x):
            stats = outs.tile([128, 6], F32, tag="stats")
            nc.vector.bn_stats(out=stats[:], in_=x[:])
            mv = outs.tile([128, 2], F32, tag="mv")
            nc.vector.bn_aggr(out=mv[:], in_=stats[:])
            return it, x, mv

        def emit_tail2(it, x, mv):
            std = outs.tile([128, 1], F32, tag="std")
            nc.scalar.activation(std[:], mv[:, 1:2],
                                 mybir.ActivationFunctionType.Sqrt,
                                 bias=epsb[:], scale=1.0)
            rstd = outs.tile([128, 1], F32, tag="rstd")
            nc.vector.reciprocal(rstd[:], std[:])
            z = outs.tile([128, H], F32, tag="z")
            nc.gpsimd.tensor_scalar(out=z[:], in0=x[:], scalar1=mv[:, 0:1],
                                    scalar2=rstd[:],
                                    op0=mybir.AluOpType.subtract,
                                    op1=mybir.AluOpType.mult)
            nc.gpsimd.tensor_tensor(out=z[:], in0=z[:], in1=gammab[:],
                                    op=mybir.AluOpType.mult)
            nc.gpsimd.tensor_tensor(out=y_r[:, it * H:(it + 1) * H],
                                    in0=z[:], in1=betab[:],
                                    op=mybir.AluOpType.add)

        STORE_T = 2

        def emit_store(s):
            t0 = s * STORE_T
            src = bass.AP(
                tensor=y_r[:].tensor, offset=y_r[:].offset + t0 * H,
                ap=[y_r[:].ap[0], [H, STORE_T], [1, H]])
            dst = bass.AP(
                tensor=out_ap.tensor, offset=out_ap.offset + t0 * 128 * H,
                ap=[[H, 128], [128 * H, STORE_T], [1, H]])
            nc.sync.dma_start(dst, src)

        pending = None
        pending2 = None

        for it in range(n_tiles):
            efsi_t = efsi_slice(it)
            ef = efsi_t[:, 0:H]
            sh = efsi_t[:, H:H + 4]
            src = efsi_t[:, H + 4:H + 4 + RES_DIM]
            dst = efsi_t[:, H + 4 + RES_DIM:EFSI_W]
            efTs = eft_slice(it)

            # --- FFN1, transposed out: rhT = relu(W1.T @ efT + b1) ---
            rhT_ps = ps_rh.tile([128, 128], F32, tag="psr")
            nc.tensor.matmul(rhT_ps[:], lhsT=w1sb[:], rhs=efTs,
                             start=True, stop=True)
            rhT = mids.tile([128, 128], BF16)
            nc.scalar.activation(rhT[:], rhT_ps[:],
                                 mybir.ActivationFunctionType.Relu,
                                 bias=b1sb[:], scale=1.0)

            # previous tile's x assembly goes first in the Pool queue so the
            # bn_stats emitted after this tile's scans never waits on it
            pend_x = emit_tail1_pool(*pending) if pending is not None else None

            # --- feats [128, 80] on Pool (per-partition sh scalars) ---
            feats = mids.tile([128, W_IN], F32)
            nc.gpsimd.tensor_scalar_mul(feats[:, 0:32], src[:, 0:32],
                                        sh[:, 0:1])
            nc.gpsimd.tensor_scalar_mul(feats[:, 32:64], dst[:, 0:32],
                                        sh[:, 0:1])
            for half, g, vt in ((src, slice(64, 72), "vt0"),
                                (dst, slice(72, 80), "vt1")):
                vecs = half[:, 32:RES_DIM].rearrange("p (m c) -> p m c", c=3)
                nc.gpsimd.tensor_scalar_mul(feats[:, g], vecs[:, :, 0],
                                            sh[:, 1:2])
                vtmp = mids.tile([128, 16], F32, tag=vt)
                nc.gpsimd.tensor_scalar_mul(vtmp[:, 0:8], vecs[:, :, 1],
                                            sh[:, 2:3])
                nc.gpsimd.tensor_scalar_mul(vtmp[:, 8:16], vecs[:, :, 2],
                                            sh[:, 3:4])
                nc.gpsimd.tensor_tensor(out=feats[:, g], in0=feats[:, g],
                                        in1=vtmp[:, 0:8],
                                        op=mybir.AluOpType.add)
                nc.gpsimd.tensor_tensor(out=feats[:, g], in0=feats[:, g],
                                        in1=vtmp[:, 8:16],
                                        op=mybir.AluOpType.add)

            # --- b2 seed: feats @ b2r via PE transpose of feats ---
            def emit_seed(feats, ef):
                featsT_ps = ps_small.tile([W_IN, 128], F32, tag="ps")
                nc.tensor.transpose(featsT_ps[:], feats[:], ident[:])
                featsT = mids.tile([W_IN, 128], F32, tag="featsT")
                nc.scalar.copy(featsT[:], featsT_ps[:])
                seed_ps = ps_small.tile([128, H], F32, tag="ps")
                nc.tensor.matmul(seed_ps[:], lhsT=featsT[:], rhs=b2rsb[:],
                                 start=True, stop=True)
                seedc = mids.tile([128, H], F32, tag="seedc")
                nc.scalar.copy(seedc[:], seed_ps[:])
                x_pre = outs.tile([128, H], F32, tag="x_pre")
                nc.gpsimd.tensor_tensor(out=x_pre[:], in0=ef, in1=seedc[:],
                                        op=mybir.AluOpType.add)
                return x_pre

            if not SEED_AFTER:
                x_pre = emit_seed(feats, ef)

            # --- FFN2 (f32r) + contraction scans into S ---
            S = outs.tile([128, H], F32, tag="S")
            for ci, (h0, npages) in enumerate(chunks):
                width = npages * W_IN
                w_ps = ps_w.tile([128, width], F32, tag="psw")
                col = 0
                while col < width:
                    n = min(512, width - col)
                    g = h0 * W_IN + col
                    n = min(n, ((g // blkw) + 1) * blkw - g)
                    nc.tensor.matmul(
                        w_ps[:, col:col + n], lhsT=rhT[:],
                        rhs=w2_slice(g, g + n),
                        start=True, stop=True)
                    col += n
                feats_b = bass.AP(tensor=feats[:].tensor,
                                  offset=feats[:].offset,
                                  ap=[feats[:].ap[0], [0, npages], [1, W_IN]])
                s_out = bass.AP(tensor=S[:].tensor, offset=S[:].offset + h0,
                                ap=[S[:].ap[0], [1, npages], [0, W_IN]])
                if ci == PSUM_SCAN_CI or (TILE0_PSUM and it < 3):
                    # chunk 0 scans straight from PSUM
                    nc.vector._custom_dve(
                        MUL_SCAN, out=s_out,
                        in0=w_ps[:].rearrange("p (s n) -> p s n", n=W_IN),
                        in1=feats_b, s0=0.0)
                else:
                    # Act evacuates to SBUF bf16; scan reads SBUF
                    w_sb = wsb.tile([128, width], BF16, tag=f"wsb{ci}")
                    nc.scalar.copy(w_sb[:], w_ps[:])
                    nc.vector._custom_dve(
                        MUL_SCAN, out=s_out,
                        in0=w_sb[:].rearrange("p (s n) -> p s n", n=W_IN),
                        in1=feats_b, s0=0.0)

            if SEED_AFTER:
                x_pre = emit_seed(feats, ef)
            if pend_x is not None:
                t1 = emit_tail1_dve(*pend_x)
                if pending2 is not None:
                    emit_tail2(*pending2)
                    if pending2[0] % STORE_T == STORE_T - 1:
                        emit_store(pending2[0] // STORE_T)
                pending2 = t1
            pending = (it, S, x_pre)

        if pending is not None:
            if pending2 is not None:
                emit_tail2(*pending2)
            t1 = emit_tail1_dve(*emit_tail1_pool(*pending))
            emit_tail2(*t1)
            emit_store(n_tiles // STORE_T - 1)

    nc.compile()
    return nc


_NC_CACHE = None


def _get_nc():
    global _NC_CACHE
    if _NC_CACHE is None:
        _NC_CACHE = _build_kernel()
    return _NC_CACHE


def _host_prep(inputs):
    import ml_dtypes
    bf16 = ml_dtypes.bfloat16
    ef = np.asarray(inputs["edge_features"], np.float32)
    sh = np.asarray(inputs["edge_sh"], np.float32).copy()
    sh[:, 1:4] /= np.float32(np.sqrt(3.0))
    idx = np.asarray(inputs["edge_index"])
    res = np.ascontiguousarray(np.asarray(inputs["res_features"], np.float32))
    w1 = np.ascontiguousarray(
        np.asarray(inputs["W1"], np.float32).astype(bf16))
    b1 = np.ascontiguousarray(
        np.asarray(inputs["b1"], np.float32).reshape(128, 1))
    scale = np.float32(1.0 / np.sqrt(80.0))
    w2 = np.asarray(inputs["W2"], np.float32) * scale
    # h-major permutation: col h*80+d = w2[:, d*64+h]
    w2 = np.ascontiguousarray(
        w2.reshape(128, W_IN, H).transpose(0, 2, 1).reshape(128, W_NUMEL)
        .astype(bf16))
    b2r = np.ascontiguousarray(
        (np.asarray(inputs["b2"], np.float32) * scale).reshape(W_IN, H))
    gb = np.ascontiguousarray(np.stack([
        np.asarray(inputs["gamma"], np.float32),
        np.asarray(inputs["beta"], np.float32)]))
    # host-side gather of endpoint node rows (marshaling, like the transposes)
    src = res[idx[1]]   # [E, 56]
    dst = res[idx[0]]   # [E, 56]

    in_maps = []
    for c in range(N_CORES):
        rows = slice(c * E_CORE, (c + 1) * E_CORE)
        efsi = np.concatenate(
            [ef[rows], sh[rows], src[rows], dst[rows]], axis=1)
        in_maps.append(dict(
            efsi=np.ascontiguousarray(efsi),
            eft=np.ascontiguousarray(ef[rows].T.astype(bf16)),
            w1=w1, b1=b1, w2=w2, b2r=b2r, gb=gb,
        ))
    return in_maps


def kernel(**inputs) -> np.ndarray:
    assert inputs["edge_features"].shape == (E_TOTAL, H)
    nc = _get_nc()
    in_maps = _host_prep(inputs)
    res = run_bass_kernel_spmd(nc, in_maps, core_ids=list(range(N_CORES)))
    return np.concatenate([r["out"] for r in res.results], axis=0)


# revision 57
# speedup vs baseline: 1.0041x; 1.0041x over previous
"""EdgeUpdate (gnn_message_passing) Trainium2 Bass kernel — 8 NeuronCores.

Contract: kernel(**inputs) takes the FULL inputs of reference.setup_inputs()
and returns the FULL [32768, 64] float32 output.

Strategy (edge-parallel SPMD):
- Edges are sharded 8 ways (4096 edges/core); node features and all
  FFN/LN parameters are replicated. One NEFF runs on cores 0-7 with
  per-core input bindings. No cross-core communication.

Per-core kernel (tiles of 128 edges, edges on partitions). DVE is the
bottleneck engine (~89% busy: the per-edge tensor-product contraction
runs as fused multiply-scan passes over the FFN2 output), so everything
else is kept off DVE and off its critical path:
- All inputs are SBUF-resident, loaded up front: tile-0-critical slices
  first on the sync queue (efsi tile 0, W1, eft cols 0:128, then b1),
  W2 block 0 concurrently on the Act HWDGE queue, bulk after. Residents
  are split into per-load-group tiles because dependency tracking is
  per-tile — one big tile would stall tile 0 on the bulk DMA. Outputs
  accumulate in a resident buffer and ship in 2-tile stores (small
  groups keep the final store off the drain path).
- The Act function table is warmed by a dummy Sqrt at t=0 so tile 0's
  relu doesn't wait ~2.6us for the table load.
- FFN1 computed transposed (rhT = relu(W1.T @ efT + b1)), bias per
  partition; eft is host-pretransposed.
- FFN2 (per-edge weight generation) in float32r into 16-h-page PSUM
  chunks, W2 host-permuted h-major.
- Chunks 0-2 are evacuated PSUM->SBUF bf16 by the Act engine; the DVE
  MUL_SCAN (running prefix of Src0*Src1, page-end sampled via a stride-0
  output dim) then reads SBUF (58-cycle access vs 120 for PSUM). Chunk 3
  scans straight from PSUM (Act can't absorb a 4th evacuation), as does
  all of tile 0 (no evac dependency -> DVE starts ~4us earlier).
- feats build on Pool (tensor_scalar with per-partition sh scalars).
- Tails are software-pipelined: the Pool x-assembly of tile t-1 is
  emitted before tile t's feats; bn_stats/bn_aggr (DVE) after tile t's
  scans; sqrt (Act) + reciprocal + LN affine (Pool) one tile later
  still. GPSIMD/Pool cannot touch PSUM, so the featsT/seed evacuations
  stay on Act.
"""

from contextlib import ExitStack

import numpy as np

import concourse.bass as bass
import concourse.dve_ops as dve_ops
import concourse.tile as tile
from concourse import bacc, mybir
from concourse.bass_utils import run_bass_kernel_spmd
from concourse.dve_spec import AluOp, C0, Spec, Src0, Src1, lower, scan
from concourse.dve_uop import DveOpSpec
from concourse.masks import make_identity

F32 = mybir.dt.float32
F32R = mybir.dt.float32r
BF16 = mybir.dt.bfloat16
N_CORES = 8
E_TOTAL = 32768
E_CORE = E_TOTAL // N_CORES
N_NODES = 16384
RES_DIM = 56
H = 64
W_IN = 80
W_NUMEL = 5120
EPS = 1e-5
CHUNK_PAGES = 16  # h-pages (of 80 values) per PSUM chunk
EFSI_W = H + 4 + 2 * RES_DIM  # 180


def _register_mul_scan():
    name = "MUL_SCAN_ANT"
    for op in dve_ops.OPS:
        if op.name == name:
            return op
    spec = Spec(
        body=scan(AluOp.ADD, Src0 * Src1, init=C0),
        reference=lambda in0, in1, s0, *a: (
            np.asarray(s0, np.float32)
            + np.cumsum(
                (in0.reshape(in0.shape[0], -1).astype(np.float32)
                 * in1.reshape(in1.shape[0], -1).astype(np.float32)),
                axis=-1)).astype(np.float32),
    )
    opcode = dve_ops._CUSTOM_DVE_ROW_BASE + len(dve_ops.OPS)
    shas = {}
    for ver in ("v3", "v4"):
        shas[ver] = DveOpSpec(name=name, opcode=opcode,
                              uops=lower(spec, ver=ver), rd1_en=True).sha(ver)
    op = dve_ops.DveOp(name, spec, subdim=False, uops_sha=shas)
    dve_ops.OPS.append(op)
    dve_ops.CUSTOM_DVE_SPECS[name] = spec
    dve_ops._SUB_OPCODE_FOR_NAME[name] = opcode
    return op


MUL_SCAN = _register_mul_scan()


PE_WARM = 8        # dummy matmuls at t=0 to ramp the PE p-state
PSUM_SCAN_CI = 3   # which chunk scans straight from PSUM (no Act evac)
SEED_AFTER = True  # emit the b2-seed path after the chunk loop
TILE0_PSUM = True  # tile 0 scans all chunks from PSUM (fast start)


def _build_kernel():
    n_tiles = E_CORE // 128
    chunks = [(0, 16), (16, 16), (32, 16), (48, 16)]

    nc = bacc.Bacc("TRN2", target_bir_lowering=False, debug=False,
                   enable_asserts=False, num_devices=N_CORES)

    # efsi row: [ef 64 | sh 4 | src 56 | dst 56] (src/dst host-gathered)
    efsi_ap = nc.dram_tensor("efsi", [E_CORE, EFSI_W], F32,
                             kind="ExternalInput").ap()
    eft_ap = nc.dram_tensor("eft", [H, E_CORE], BF16, kind="ExternalInput").ap()
    w1_ap = nc.dram_tensor("w1", [H, 128], BF16, kind="ExternalInput").ap()
    b1_ap = nc.dram_tensor("b1", [128, 1], F32, kind="ExternalInput").ap()
    w2_ap = nc.dram_tensor("w2", [128, W_NUMEL], BF16, kind="ExternalInput").ap()
    b2r_ap = nc.dram_tensor("b2r", [W_IN, H], F32, kind="ExternalInput").ap()
    gb_ap = nc.dram_tensor("gb", [2, H], F32, kind="ExternalInput").ap()
    out_ap = nc.dram_tensor("out", [E_CORE, H], F32, kind="ExternalOutput").ap()

    with tile.TileContext(nc) as tc, ExitStack() as ctx:
        singles = ctx.enter_context(tc.tile_pool(name="singles", bufs=1))
        mids = ctx.enter_context(tc.tile_pool(name="mids", bufs=3))
        wsb = ctx.enter_context(tc.tile_pool(name="wsb", bufs=2))
        outs = ctx.enter_context(tc.tile_pool(name="outs", bufs=3))
        ps_small = ctx.enter_context(
            tc.tile_pool(name="ps_small", bufs=1, space="PSUM"))
        ps_rh = ctx.enter_context(
            tc.tile_pool(name="ps_rh", bufs=1, space="PSUM"))
        ps_w = ctx.enter_context(tc.tile_pool(name="ps_w", bufs=2, space="PSUM"))

        # --- resident constants + inputs (all DMAs on the sync queue) ---
        # Warm the activation function table immediately (Sqrt forces the
        # sqrt_and_others set, which also holds Relu/Copy) so tile 0's relu
        # doesn't wait ~2.6us for the table load.
        epsb = singles.tile([128, 1], F32)
        nc.vector.memset(epsb[:], EPS)
        actwarm = singles.tile([128, 1], F32)
        nc.scalar.activation(actwarm[:], epsb[:],
                             mybir.ActivationFunctionType.Sqrt)
        # PE p-state warmup: dummy matmuls on memset data ramp the PE to
        # full clock before tile 0's real matmuls arrive
        wdum = singles.tile([128, 128], BF16)
        nc.gpsimd.memset(wdum[:], 0.0)
        wdum_b = bass.AP(tensor=wdum[:].tensor, offset=wdum[:].offset,
                         ap=[wdum[:].ap[0], [0, 4], [1, 128]])
        for _ in range(PE_WARM):
            warm_ps = ps_small.tile([128, 512], F32, tag="ps")
            nc.tensor.matmul(warm_ps[:], lhsT=wdum[:], rhs=wdum_b,
                             start=True, stop=True)

        w1sb = singles.tile([H, 128], BF16)
        b1sb = singles.tile([128, 1], F32)

        # Residents are split per load-group: the tile framework tracks
        # dependencies per-tile, so a single big tile would stall tile 0 on
        # the bulk DMA.
        EFSI_GROUPS = [(0, 1), (1, 4), (4, n_tiles)]
        EFT_GROUPS = [(0, 128), (128, 1024), (1024, E_CORE)]
        efsi_tiles = [singles.tile([128, (t1 - t0) * EFSI_W], F32,
                                   name=f"efsi{t0}", tag=f"efsi{t0}")
                      for t0, t1 in EFSI_GROUPS]
        eft_tiles = [singles.tile([H, c1 - c0], BF16,
                                  name=f"eft{c0}", tag=f"eft{c0}")
                     for c0, c1 in EFT_GROUPS]

        def efsi_slice(t):
            for (t0, t1), tl in zip(EFSI_GROUPS, efsi_tiles):
                if t0 <= t < t1:
                    return tl[:, (t - t0) * EFSI_W:(t - t0 + 1) * EFSI_W]
            raise AssertionError

        def eft_slice(t):
            for (c0, c1), tl in zip(EFT_GROUPS, eft_tiles):
                if c0 <= t * 128 < c1:
                    return tl[:, t * 128 - c0:(t + 1) * 128 - c0]
            raise AssertionError

        def efsi_load(gi):
            (t0, t1), tl = EFSI_GROUPS[gi], efsi_tiles[gi]
            dst = bass.AP(
                tensor=tl[:].tensor, offset=tl[:].offset,
                ap=[tl[:].ap[0], [EFSI_W, t1 - t0], [1, EFSI_W]])
            src = bass.AP(
                tensor=efsi_ap.tensor,
                offset=efsi_ap.offset + t0 * 128 * EFSI_W,
                ap=[[EFSI_W, 128], [128 * EFSI_W, t1 - t0], [1, EFSI_W]])
            nc.sync.dma_start(dst, src)

        w2blocks = []
        nblk = 4
        blkw = W_NUMEL // nblk

        def w2_load(bi, eng):
            w2b = singles.tile([128, blkw], BF16, tag=f"w2b{bi}")
            eng.dma_start(w2b[:], w2_ap[:, bi * blkw:(bi + 1) * blkw])
            w2blocks.append(w2b)

        # tile-0-critical loads first on the sync queue; w2b0 on the Act
        # queue — the two HWDGE queues run concurrently.
        efsi_load(0)
        nc.sync.dma_start(w1sb[:], w1_ap[:])
        nc.sync.dma_start(eft_tiles[0][:], eft_ap[:, 0:128])
        w2_load(0, nc.scalar)
        nc.sync.dma_start(b1sb[:], b1_ap[:])
        for bi in range(1, nblk):
            w2_load(bi, nc.sync)
        efsi_load(1)
        nc.sync.dma_start(eft_tiles[1][:], eft_ap[:, 128:1024])
        nc.sync.dma_start(eft_tiles[2][:], eft_ap[:, 1024:])
        efsi_load(2)

        b2rsb = singles.tile([W_IN, H], F32)
        nc.sync.dma_start(b2rsb[:], b2r_ap[:])
        gammab = singles.tile([128, H], F32)
        nc.sync.dma_start(gammab[:], bass.AP(
            tensor=gb_ap.tensor, offset=gb_ap.offset,
            ap=[[0, 128]] + gb_ap[0:1, :].ap[1:]))
        betab = singles.tile([128, H], F32)
        nc.sync.dma_start(betab[:], bass.AP(
            tensor=gb_ap.tensor, offset=gb_ap.offset + H,
            ap=[[0, 128]] + gb_ap[1:2, :].ap[1:]))
        ident = singles.tile([128, 128], F32)
        make_identity(nc, ident[:])
        y_r = singles.tile([128, n_tiles * H], F32)

        def w2_slice(c0, c1):
            bi = c0 // blkw
            assert c1 <= (bi + 1) * blkw
            return w2blocks[bi][:, c0 - bi * blkw:c1 - bi * blkw]

        def emit_tail1_pool(it, S, x_pre):
            # T = per-page sums from the unchained prefix samples: diff along
            # h, then add back the previous chunk's final prefix at the 3
            # chunk-boundary columns.
            T = outs.tile([128, H], F32, tag="T")
            nc.gpsimd.tensor_copy(T[:, 0:1], S[:, 0:1])
            nc.gpsimd.tensor_tensor(out=T[:, 1:], in0=S[:, 1:],
                                    in1=S[:, :H - 1],
                                    op=mybir.AluOpType.subtract)
            tb = bass.AP(tensor=T[:].tensor, offset=T[:].offset + CHUNK_PAGES,
                         ap=[T[:].ap[0], [CHUNK_PAGES, 3]])
            sb = bass.AP(tensor=S[:].tensor,
                         offset=S[:].offset + CHUNK_PAGES - 1,
                         ap=[S[:].ap[0], [CHUNK_PAGES, 3]])
            nc.gpsimd.tensor_tensor(out=tb, in0=tb, in1=sb,
                                    op=mybir.AluOpType.add)
            x = outs.tile([128, H], F32, tag="x")
            nc.gpsimd.tensor_tensor(out=x[:], in0=x_pre[:], in1=T[:],
                                    op=mybir.AluOpType.add)
            return it, x

        def emit_tail1_dve(it, x):
            stats = outs.tile([128, 6], F32, tag="stats")
            nc.vector.bn_stats(out=stats[:], in_=x[:])
            mv = outs.tile([128, 2], F32, tag="mv")
            nc.vector.bn_aggr(out=mv[:], in_=stats[:])
            return it, x, mv

        def emit_tail2(it, x, mv):
            std = outs.tile([128, 1], F32, tag="std")
            nc.scalar.activation(std[:], mv[:, 1:2],
                                 mybir.ActivationFunctionType.Sqrt,
                                 bias=epsb[:], scale=1.0)
            rstd = outs.tile([128, 1], F32, tag="rstd")
            nc.vector.reciprocal(rstd[:], std[:])
            z = outs.tile([128, H], F32, tag="z")
            nc.gpsimd.tensor_scalar(out=z[:], in0=x[:], scalar1=mv[:, 0:1],
                                    scalar2=rstd[:],
                                    op0=mybir.AluOpType.subtract,
                                    op1=mybir.AluOpType.mult)
            nc.gpsimd.tensor_tensor(out=z[:], in0=z[:], in1=gammab[:],
                                    op=mybir.AluOpType.mult)
            nc.gpsimd.tensor_tensor(out=y_r[:, it * H:(it + 1) * H],
                                    in0=z[:], in1=betab[:],
                                    op=mybir.AluOpType.add)

        STORE_T = 2

        def emit_store(s):
            t0 = s * STORE_T
            src = bass.AP(
                tensor=y_r[:].tensor, offset=y_r[:].offset + t0 * H,
                ap=[y_r[:].ap[0], [H, STORE_T], [1, H]])
            dst = bass.AP(
                tensor=out_ap.tensor, offset=out_ap.offset + t0 * 128 * H,
                ap=[[H, 128], [128 * H, STORE_T], [1, H]])
            nc.sync.dma_start(dst, src)

        pending = None
        pending2 = None

        for it in range(n_tiles):
            efsi_t = efsi_slice(it)
            ef = efsi_t[:, 0:H]
            sh = efsi_t[:, H:H + 4]
            src = efsi_t[:, H + 4:H + 4 + RES_DIM]
            dst = efsi_t[:, H + 4 + RES_DIM:EFSI_W]
            efTs = eft_slice(it)

            # --- FFN1, transposed out: rhT = relu(W1.T @ efT + b1) ---
            rhT_ps = ps_rh.tile([128, 128], F32, tag="psr")
            nc.tensor.matmul(rhT_ps[:], lhsT=w1sb[:], rhs=efTs,
                             start=True, stop=True)
            rhT = mids.tile([128, 128], BF16)
            nc.scalar.activation(rhT[:], rhT_ps[:],
                                 mybir.ActivationFunctionType.Relu,
                                 bias=b1sb[:], scale=1.0)

            # previous tile's x assembly goes first in the Pool queue so the
            # bn_stats emitted after this tile's scans never waits on it
            pend_x = emit_tail1_pool(*pending) if pending is not None else None

            # --- feats [128, 80] on Pool (per-partition sh scalars) ---
            feats = mids.tile([128, W_IN], F32)
            nc.gpsimd.tensor_scalar_mul(feats[:, 0:32], src[:, 0:32],
                                        sh[:, 0:1])
            nc.gpsimd.tensor_scalar_mul(feats[:, 32:64], dst[:, 0:32],
                                        sh[:, 0:1])
            for half, g, vt in ((src, slice(64, 72), "vt0"),
                                (dst, slice(72, 80), "vt1")):
                vecs = half[:, 32:RES_DIM].rearrange("p (m c) -> p m c", c=3)
                nc.gpsimd.tensor_scalar_mul(feats[:, g], vecs[:, :, 0],
                                            sh[:, 1:2])
                vtmp = mids.tile([128, 16], F32, tag=vt)
                nc.gpsimd.tensor_scalar_mul(vtmp[:, 0:8], vecs[:, :, 1],
                                            sh[:, 2:3])
                nc.gpsimd.tensor_scalar_mul(vtmp[:, 8:16], vecs[:, :, 2],
                                            sh[:, 3:4])
                nc.gpsimd.tensor_tensor(out=feats[:, g], in0=feats[:, g],
                                        in1=vtmp[:, 0:8],
                                        op=mybir.AluOpType.add)
                nc.gpsimd.tensor_tensor(out=feats[:, g], in0=feats[:, g],
                                        in1=vtmp[:, 8:16],
                                        op=mybir.AluOpType.add)

            # --- b2 seed: feats @ b2r via PE transpose of feats ---
            def emit_seed(feats, ef):
                featsT_ps = ps_small.tile([W_IN, 128], F32, tag="ps")
                nc.tensor.transpose(featsT_ps[:], feats[:], ident[:])
                featsT = mids.tile([W_IN, 128], F32, tag="featsT")
                nc.scalar.copy(featsT[:], featsT_ps[:])
                seed_ps = ps_small.tile([128, H], F32, tag="ps")
                nc.tensor.matmul(seed_ps[:], lhsT=featsT[:], rhs=b2rsb[:],
                                 start=True, stop=True)
                seedc = mids.tile([128, H], F32, tag="seedc")
                nc.scalar.copy(seedc[:], seed_ps[:])
                x_pre = outs.tile([128, H], F32, tag="x_pre")
                nc.gpsimd.tensor_tensor(out=x_pre[:], in0=ef, in1=seedc[:],
                                        op=mybir.AluOpType.add)
                return x_pre

            if not SEED_AFTER:
                x_pre = emit_seed(feats, ef)

            # --- FFN2 (f32r) + contraction scans into S ---
            S = outs.tile([128, H], F32, tag="S")
            for ci, (h0, npages) in enumerate(chunks):
                width = npages * W_IN
                w_ps = ps_w.tile([128, width], F32, tag="psw")
                col = 0
                while col < width:
                    n = min(512, width - col)
                    g = h0 * W_IN + col
                    n = min(n, ((g // blkw) + 1) * blkw - g)
                    nc.tensor.matmul(
                        w_ps[:, col:col + n], lhsT=rhT[:],
                        rhs=w2_slice(g, g + n),
                        start=True, stop=True)
                    col += n
                feats_b = bass.AP(tensor=feats[:].tensor,
                                  offset=feats[:].offset,
                                  ap=[feats[:].ap[0], [0, npages], [1, W_IN]])
                s_out = bass.AP(tensor=S[:].tensor, offset=S[:].offset + h0,
                                ap=[S[:].ap[0], [1, npages], [0, W_IN]])
                if ci == PSUM_SCAN_CI or (TILE0_PSUM and it < 3):
                    # chunk 0 scans straight from PSUM
                    nc.vector._custom_dve(
                        MUL_SCAN, out=s_out,
                        in0=w_ps[:].rearrange("p (s n) -> p s n", n=W_IN),
                        in1=feats_b, s0=0.0)
                else:
                    # Act evacuates to SBUF bf16; scan reads SBUF
                    w_sb = wsb.tile([128, width], BF16, tag=f"wsb{ci}")
                    nc.scalar.copy(w_sb[:], w_ps[:])
                    nc.vector._custom_dve(
                        MUL_SCAN, out=s_out,
                        in0=w_sb[:].rearrange("p (s n) -> p s n", n=W_IN),
                        in1=feats_b, s0=0.0)

            if SEED_AFTER:
                x_pre = emit_seed(feats, ef)
            if pend_x is not None:
                t1 = emit_tail1_dve(*pend_x)
                if pending2 is not None:
                    emit_tail2(*pending2)
                    if pending2[0] % STORE_T == STORE_T - 1:
                        emit_store(pending2[0] // STORE_T)
                pending2 = t1
            pending = (it, S, x_pre)

        if pending is not None:
            if pending2 is not None:
                emit_tail2(*pending2)
            t1 = emit_tail1_dve(*emit_tail1_pool(*pending))
            emit_tail2(*t1)
            emit_store(n_tiles // STORE_T - 1)

    nc.compile()
    return nc


_NC_CACHE = None


def _get_nc():
    global _NC_CACHE
    if _NC_CACHE is None:
        _NC_CACHE = _build_kernel()
    return _NC_CACHE


def _host_prep(inputs):
    import ml_dtypes
    bf16 = ml_dtypes.bfloat16
    ef = np.asarray(inputs["edge_features"], np.float32)
    sh = np.asarray(inputs["edge_sh"], np.float32).copy()
    sh[:, 1:4] /= np.float32(np.sqrt(3.0))
    idx = np.asarray(inputs["edge_index"])
    res = np.ascontiguousarray(np.asarray(inputs["res_features"], np.float32))
    w1 = np.ascontiguousarray(
        np.asarray(inputs["W1"], np.float32).astype(bf16))
    b1 = np.ascontiguousarray(
        np.asarray(inputs["b1"], np.float32).reshape(128, 1))
    scale = np.float32(1.0 / np.sqrt(80.0))
    w2 = np.asarray(inputs["W2"], np.float32) * scale
    # h-major permutation: col h*80+d = w2[:, d*64+h]
    w2 = np.ascontiguousarray(
        w2.reshape(128, W_IN, H).transpose(0, 2, 1).reshape(128, W_NUMEL)
        .astype(bf16))
    b2r = np.ascontiguousarray(
        (np.asarray(inputs["b2"], np.float32) * scale).reshape(W_IN, H))
    gb = np.ascontiguousarray(np.stack([
        np.asarray(inputs["gamma"], np.float32),
        np.asarray(inputs["beta"], np.float32)]))
    # host-side gather of endpoint node rows (marshaling, like the transposes)
    src = res[idx[1]]   # [E, 56]
    dst = res[idx[0]]   # [E, 56]

    in_maps = []
    for c in range(N_CORES):
        rows = slice(c * E_CORE, (c + 1) * E_CORE)
        efsi = np.concatenate(
            [ef[rows], sh[rows], src[rows], dst[rows]], axis=1)
        in_maps.append(dict(
            efsi=np.ascontiguousarray(efsi),
            eft=np.ascontiguousarray(ef[rows].T.astype(bf16)),
            w1=w1, b1=b1, w2=w2, b2r=b2r, gb=gb,
        ))
    return in_maps


def kernel(**inputs) -> np.ndarray:
    assert inputs["edge_features"].shape == (E_TOTAL, H)
    nc = _get_nc()
    in_maps = _host_prep(inputs)
    res = run_bass_kernel_spmd(nc, in_maps, core_ids=list(range(N_CORES)))
    return np.concatenate([r["out"] for r in res.results], axis=0)
